# revision 25
# baseline (speedup 1.0000x reference)
"""Trainium2 Bass kernel for nn_MoEPolicy (moe_routing).

Strategy (8 NeuronCores, SPMD, no collectives):
  - 32 graphs -> 4 graphs per core; each graph padded to 768 node slots
    (3072 padded node slots per core). Nodes within a graph are assigned to
    its 6 128-node windows balancing edge counts.
  - Kernel 1 (per core): c_emb (replicated), edge aggregation via
    dma_gather + one-hot PSUM matmuls, v_emb (relu+LN), struct-token
    attention, masked pooling, gating logits.
  - Host: top-4 expert selection per graph from device-computed logits
    (index selection only), slices expert weights per core.
  - Kernel 2 (per core): route weights on device, 4 dedicated experts per
    graph + 2 shared experts (exact: skipped experts have exactly zero
    route weight), LN via mean-centering folded into W2 (device-computed
    W2 @ P), combine, task head.
All floating-point math runs on device; the host only shards, pads,
permutes, and selects indices.
"""

import sys

for _p in ("/opt/trn_rl_repo",):
    if _p not in sys.path:
        sys.path.insert(0, _p)

import numpy as np

import concourse.bacc as bacc
import concourse.mybir as mybir
import concourse.tile as tile
from concourse.bass_utils import run_bass_kernel_spmd

F32 = mybir.dt.float32
I16 = mybir.dt.int16
AF = mybir.ActivationFunctionType
ALU = mybir.AluOpType

# problem constants
D = 128
TD = 128
T = 64
NE = 16
KS = 2
TOPK = 4
TEMP = 0.6
B = 32
M = 10000
N = 20000
E = 160000
CF, VF, EF = 4, 6, 1

NCORE = 8
GPC = B // NCORE            # graphs per core
PAD_G = 768                 # node slots per graph
NC_NODES = GPC * PAD_G      # 3072
WPG = PAD_G // 128          # windows per graph
NWIN = GPC * WPG            # 24 windows per core
LN_EPS = 1e-5
ISQ_TD = 1.0 / float(np.sqrt(np.float32(TD)))

CORE_IDS = list(range(NCORE))


# ---------------------------------------------------------------- host plan

def _plan(edge_cons, edge_vars, edge_attr, batch_idx):
    """Node slot assignment + edge window schedule. Pure index work."""
    order = np.argsort(batch_idx, kind="stable")
    bs = batch_idx[order]
    deg = np.bincount(edge_vars, minlength=N)

    node_of_slot = -np.ones((NCORE, NC_NODES), dtype=np.int64)
    slot_of_node = np.empty(N, dtype=np.int64)       # global slot = core*NC + s
    counts = np.zeros((NCORE, GPC), dtype=np.int64)  # real nodes per graph

    for g in range(B):
        nodes = order[np.searchsorted(bs, g, side="left"):
                      np.searchsorted(bs, g, side="right")]
        core, lg = g // GPC, g % GPC
        counts[core, lg] = len(nodes)
        if len(nodes) > PAD_G:
            raise RuntimeError(f"graph {g} has {len(nodes)} nodes > PAD_G={PAD_G}")
        # balance edge load across the graph's WPG windows
        nds = nodes[np.argsort(-deg[nodes], kind="stable")]
        wload = np.zeros(WPG, dtype=np.int64)
        wfill = np.zeros(WPG, dtype=np.int64)
        base = lg * PAD_G
        for nd in nds:
            cand = np.where(wfill < 128)[0]
            w = cand[np.argmin(wload[cand])]
            s = base + w * 128 + wfill[w]
            node_of_slot[core, s] = nd
            slot_of_node[nd] = core * NC_NODES + s
            wload[w] += deg[nd]
            wfill[w] += 1

    # edges -> (core, window, lane j)
    eslot = slot_of_node[edge_vars]
    ecore = eslot // NC_NODES
    es = eslot % NC_NODES
    ewin = es // 128
    ej = es % 128

    # tiles per window position, shared across cores
    cw = np.zeros((NCORE, NWIN), dtype=np.int64)
    per = {}
    for c in range(NCORE):
        sel = np.where(ecore == c)[0]
        for w in range(NWIN):
            ews = sel[ewin[sel] == w]
            per[(c, w)] = ews
            cw[c, w] = max(1, -(-len(ews) // 128))
    CW = cw.max(axis=0)
    ntot = int(CW.sum())

    ecidx = np.zeros((NCORE, 128 * ntot), dtype=np.int64)   # cons index per slot
    used = np.zeros((NCORE, 128 * ntot), dtype=bool)
    vloc = np.full((NCORE, 128 * ntot), -1.0, dtype=np.float32)
    eav = np.zeros((NCORE, 128 * ntot), dtype=np.float32)
    offs = np.concatenate([[0], np.cumsum(CW)]) * 128
    ea_flat = edge_attr.reshape(-1).astype(np.float32)
    for c in range(NCORE):
        for w in range(NWIN):
            ews = per[(c, w)]
            o = offs[w]
            ecidx[c, o:o + len(ews)] = edge_cons[ews]
            used[c, o:o + len(ews)] = True
            vloc[c, o:o + len(ews)] = ej[ews]
            eav[c, o:o + len(ews)] = ea_flat[ews]

    return dict(node_of_slot=node_of_slot, counts=counts, CW=CW.tolist(),
                ntot=ntot, ecidx=ecidx, used=used, vloc=vloc, eav=eav)


# ------------------------------------------------------------- build kernel1

def _build_k1(CW, skip_bc, skip_be):
    ntot = int(sum(CW))
    nc = bacc.Bacc("TRN2", target_bir_lowering=False, debug=False,
                   num_devices=NCORE)

    def din(name, shape, dt=F32):
        return nc.dram_tensor(name, shape, dt, kind="ExternalInput")

    edgecf = din("edgecf", [128, ntot * (CF + 1)])
    Wc_aug = din("Wc_aug", [CF + 1, D])
    Wv = din("Wv", [VF, D])
    bv_col = din("bv_col", [D, 1])
    vfeatT = din("vfeatT", [VF, NC_NODES])
    We_col = din("We_col", [D, 1])
    be_col = din("be_col", [D, 1])
    lng_col = din("lng_col", [D, 1])
    lnb_col = din("lnb_col", [D, 1])
    Wq_i = din("Wq", [D, TD])
    bq_col = din("bq_col", [TD, 1])
    tokKT = din("tokKT", [TD, T])
    tokV_i = din("tokV", [T, TD])
    Wg_r = din("Wg_r", [D, 2, NE])
    bg_col = din("bg_col", [NE, 1])
    eb_col = din("eb_col", [NE, 1])
    alpha11 = din("alpha11", [1, 1])
    iota_i = din("iota", [128, 128])
    ident_i = din("ident", [128, 128])
    P_i = din("P_mat", [128, 128])
    onesr_i = din("onesr", [1, 512])
    vloc_i = din("vloc", [128, ntot])
    eav_i = din("eav", [128, ntot])
    invc_i = din("invcnt", [128, GPC])
    padc_i = din("padcnt", [128, GPC])

    vembT_o = nc.dram_tensor("vembT", [D, NC_NODES], F32, kind="ExternalOutput")
    logitsT_o = nc.dram_tensor("logitsT", [NE, GPC], F32, kind="ExternalOutput")

    with tile.TileContext(nc) as tc:
        with (
            tc.tile_pool(name="const", bufs=1) as cp,
            tc.tile_pool(name="oh", bufs=4) as ohp,
            tc.tile_pool(name="wk", bufs=3) as wk,
            tc.tile_pool(name="sm", bufs=4) as smp,
            tc.tile_pool(name="pT0", bufs=1, space="PSUM") as pT0p,
            tc.tile_pool(name="pT1", bufs=2, space="PSUM") as pT1p,
            tc.tile_pool(name="pG1", bufs=2, space="PSUM") as pG1p,
            tc.tile_pool(name="pG0", bufs=1, space="PSUM") as pG0p,
            tc.tile_pool(name="pmsc", bufs=2, space="PSUM") as pmsc,
        ):
            # ---- load constants
            _ld = [0]
            def load(ap_dram, shape, dt=F32):
                _ld[0] += 1
                t_ = cp.tile(shape, dt, tag=f"cst{_ld[0]}")
                nc.sync.dma_start(t_[:], ap_dram[:])
                return t_

            ecf_s = load(edgecf, [128, ntot * (CF + 1)])
            Wca_s = load(Wc_aug, [CF + 1, D])
            Wv_s = load(Wv, [VF, D])
            bv_s = load(bv_col, [D, 1])
            vfT_s = load(vfeatT, [VF, NC_NODES])
            We_s = load(We_col, [D, 1])
            be_s = load(be_col, [D, 1])
            lng_s = load(lng_col, [D, 1])
            lnb_s = load(lnb_col, [D, 1])
            Wq_s = load(Wq_i, [D, TD])
            bq_s = load(bq_col, [TD, 1])
            tKT_s = load(tokKT, [TD, T])
            tV_s = load(tokV_i, [T, TD])
            Wg_s = load(Wg_r, [D, 2, NE])
            bg_s = load(bg_col, [NE, 1])
            eb_s = load(eb_col, [NE, 1])
            al_s = load(alpha11, [1, 1])
            io_s = load(iota_i, [128, 128])
            id_s = load(ident_i, [128, 128])
            P_s = load(P_i, [128, 128])
            on_s = load(onesr_i, [1, 512])
            vl_s = load(vloc_i, [128, ntot])
            ea_s = load(eav_i, [128, ntot])
            ic_s = load(invc_i, [128, GPC])
            pc_s = load(padc_i, [128, GPC])
            ones_col = cp.tile([128, 1], F32)
            nc.vector.memset(ones_col[:], 1.0)
            eps11 = cp.tile([1, 1], F32)
            nc.vector.memset(eps11[:], LN_EPS)

            pbqK = pmsc.tile([1, T], F32, tag="pst")
            nc.tensor.matmul(pbqK[:], bq_s[:], tKT_s[:], start=True, stop=True)
            bqK_s = cp.tile([1, T], F32)
            nc.vector.tensor_copy(bqK_s[:], pbqK[:])

            vembT_s = cp.tile([D, NC_NODES], F32)
            wsum_s = cp.tile([D, NWIN], F32)
            nsum_s = cp.tile([D, NWIN], F32)

            offs = np.concatenate([[0], np.cumsum(CW)]).astype(int)

            for gc in range(GPC):
                for wl in range(WPG):
                    w = gc * WPG + wl
                    wt = int(CW[w])
                    ns = slice(w * 128, (w + 1) * 128)

                    CF1 = CF + 1
                    pG1 = pG1p.tile([CF1, 128], F32, tag="G1")
                    pG0 = None if skip_be else pG0p.tile([CF1, 128], F32, tag="G0")
                    for t_ in range(wt):
                        gt = int(offs[w]) + t_
                        oea = ohp.tile([128, 128], F32, tag="oea")
                        eng = nc.vector if t_ % 2 == 0 else nc.gpsimd
                        eng.tensor_scalar(
                            oea[:], io_s[:], vl_s[:, gt:gt + 1],
                            ea_s[:, gt:gt + 1], ALU.is_equal, ALU.mult)
                        nc.tensor.matmul(pG1[:], ecf_s[:, gt * CF1:(gt + 1) * CF1],
                                         oea[:], start=(t_ == 0),
                                         stop=(t_ == wt - 1))
                        if not skip_be:
                            o01 = ohp.tile([128, 128], F32, tag="o01")
                            nc.gpsimd.tensor_scalar(
                                o01[:], io_s[:], vl_s[:, gt:gt + 1], None,
                                ALU.is_equal)
                            nc.tensor.matmul(pG0[:], ecf_s[:, gt * CF1:(gt + 1) * CF1],
                                             o01[:], start=(t_ == 0),
                                             stop=(t_ == wt - 1))
                    G1_sb = wk.tile([CF1, 128], F32, tag="g1sb")
                    nc.vector.tensor_copy(G1_sb[:], pG1[:])
                    pT1 = pT1p.tile([128, 128], F32, tag="T1")
                    nc.tensor.matmul(pT1[:], Wca_s[:], G1_sb[:],
                                     start=True, stop=True)
                    if not skip_be:
                        G0_sb = wk.tile([CF1, 128], F32, tag="g0sb")
                        nc.vector.tensor_copy(G0_sb[:], pG0[:])
                        pT0 = pT0p.tile([128, 128], F32, tag="T0")
                        nc.tensor.matmul(pT0[:], Wca_s[:], G0_sb[:],
                                         start=True, stop=True)

                    pv0 = pmsc.tile([128, 128], F32, tag="pmisc")
                    nc.tensor.matmul(pv0[:], Wv_s[:], vfT_s[:, ns],
                                     start=True, stop=True)

                    # s = T1*We (+ T0*be) + v0 ; x = relu(s + bv)
                    v0_sb = wk.tile([128, 128], F32, tag="v0")
                    nc.scalar.copy(v0_sb[:], pv0[:])
                    s_sb = wk.tile([128, 128], F32, tag="s")
                    nc.vector.scalar_tensor_tensor(
                        s_sb[:], pT1[:], We_s[:], v0_sb[:], ALU.mult, ALU.add)
                    if not skip_be:
                        nc.vector.scalar_tensor_tensor(
                            s_sb[:], pT0[:], be_s[:], s_sb[:], ALU.mult, ALU.add)
                    x_sb = wk.tile([128, 128], F32, tag="x")
                    nc.scalar.activation(x_sb[:], s_sb[:], AF.Relu, bias=bv_s[:])

                    # LN feature-major: center via P, rstd via row stats
                    pc_ = pmsc.tile([128, 128], F32, tag="pmisc")
                    nc.tensor.matmul(pc_[:], P_s[:], x_sb[:], start=True, stop=True)
                    sq = wk.tile([128, 128], F32, tag="sq")
                    nc.scalar.activation(sq[:], pc_[:], AF.Square)
                    pst = pmsc.tile([1, 128], F32, tag="pst")
                    nc.tensor.matmul(pst[:], ones_col[:], sq[:], start=True, stop=True)
                    sdr = smp.tile([1, 128], F32, tag="sdr")
                    nc.scalar.activation(sdr[:], pst[:], AF.Sqrt,
                                         bias=eps11[:], scale=1.0 / D)
                    rstd = smp.tile([1, 128], F32, tag="rstd")
                    nc.vector.reciprocal(rstd[:], sdr[:])
                    cw_sb = wk.tile([128, 128], F32, tag="cw")
                    nc.vector.tensor_scalar(cw_sb[:], pc_[:], lng_s[:], None,
                                            ALU.mult)
                    pA = pmsc.tile([128, 128], F32, tag="pmisc")
                    nc.tensor.matmul(pA[:], on_s[:, :128], rstd[:],
                                     start=True, stop=True)
                    u_sb = wk.tile([128, 128], F32, tag="u")
                    nc.vector.tensor_tensor(u_sb[:], cw_sb[:], pA[:], ALU.mult)
                    nc.gpsimd.tensor_scalar(vembT_s[:, ns], u_sb[:], lnb_s[:],
                                            None, ALU.add)

                    nc.vector.tensor_reduce(wsum_s[:, w:w + 1], vembT_s[:, ns],
                                            mybir.AxisListType.X, ALU.add)

                    # struct attention
                    pq = pmsc.tile([128, 128], F32, tag="pmisc")
                    nc.tensor.matmul(pq[:], Wq_s[:], vembT_s[:, ns],
                                     start=True, stop=True)
                    q_sb = wk.tile([128, 128], F32, tag="q")
                    nc.scalar.copy(q_sb[:], pq[:])
                    psc = pmsc.tile([128, T], F32, tag="pmisc")
                    nc.tensor.matmul(psc[:], q_sb[:], tKT_s[:],
                                     start=True, stop=False)
                    nc.tensor.matmul(psc[:], on_s[:, :128], bqK_s[:],
                                     start=False, stop=True)
                    mx = smp.tile([128, 1], F32, tag="mx")
                    nc.vector.tensor_reduce(mx[:], psc[:], mybir.AxisListType.X,
                                            ALU.max)
                    mxs = smp.tile([128, 1], F32, tag="mxs")
                    nc.gpsimd.tensor_scalar(mxs[:], mx[:], -ISQ_TD, None, ALU.mult)
                    ex = wk.tile([128, T], F32, tag="ex")
                    nc.scalar.activation(ex[:], psc[:], AF.Exp,
                                         bias=mxs[:], scale=ISQ_TD)
                    sm = smp.tile([128, 1], F32, tag="sm")
                    nc.vector.tensor_reduce(sm[:], ex[:], mybir.AxisListType.X,
                                            ALU.add)
                    rc = smp.tile([128, 1], F32, tag="rc")
                    nc.vector.reciprocal(rc[:], sm[:])
                    wts = wk.tile([128, T], F32, tag="wts")
                    nc.gpsimd.tensor_scalar(wts[:], ex[:], rc[:], None, ALU.mult)
                    pwT = pmsc.tile([T, 128], F32, tag="pmisc")
                    nc.tensor.transpose(pwT[:], wts[:], id_s[:])
                    wT_sb = wk.tile([T, 128], F32, tag="wT")
                    nc.vector.tensor_copy(wT_sb[:], pwT[:])
                    pns = pmsc.tile([128, 128], F32, tag="pmisc")
                    nc.tensor.matmul(pns[:], tV_s[:], wT_sb[:], start=True, stop=True)
                    nc.vector.tensor_reduce(nsum_s[:, w:w + 1], pns[:],
                                            mybir.AxisListType.X, ALU.add)

            nc.sync.dma_start(vembT_o[:], vembT_s[:])

            # ---- pad column mini-pipeline (exact clone of per-window math)
            z0 = smp.tile([128, 1], F32, tag="z0")
            nc.vector.memset(z0[:], 0.0)
            xp = smp.tile([128, 1], F32, tag="xp")
            nc.scalar.activation(xp[:], z0[:], AF.Relu, bias=bv_s[:])
            pcp = pmsc.tile([128, 1], F32, tag="pmisc")
            nc.tensor.matmul(pcp[:], P_s[:], xp[:], start=True, stop=True)
            sqp = smp.tile([128, 1], F32, tag="sqp")
            nc.scalar.activation(sqp[:], pcp[:], AF.Square)
            pstp = pmsc.tile([1, 1], F32, tag="pst")
            nc.tensor.matmul(pstp[:], ones_col[:], sqp[:], start=True, stop=True)
            sdp = smp.tile([1, 1], F32, tag="sdp")
            nc.scalar.activation(sdp[:], pstp[:], AF.Sqrt, bias=eps11[:],
                                 scale=1.0 / D)
            rsp = smp.tile([1, 1], F32, tag="rsp")
            nc.vector.reciprocal(rsp[:], sdp[:])
            cwp = smp.tile([128, 1], F32, tag="cwp")
            nc.vector.tensor_scalar(cwp[:], pcp[:], lng_s[:], None, ALU.mult)
            pAp = pmsc.tile([128, 1], F32, tag="pmisc")
            nc.tensor.matmul(pAp[:], on_s[:, :128], rsp[:], start=True, stop=True)
            up = smp.tile([128, 1], F32, tag="up")
            nc.vector.tensor_tensor(up[:], cwp[:], pAp[:], ALU.mult)
            vp = smp.tile([128, 1], F32, tag="vp")
            nc.gpsimd.tensor_scalar(vp[:], up[:], lnb_s[:], None, ALU.add)

            pqp = pmsc.tile([128, 1], F32, tag="pmisc")
            nc.tensor.matmul(pqp[:], Wq_s[:], vp[:], start=True, stop=True)
            qp = smp.tile([128, 1], F32, tag="qp")
            nc.scalar.copy(qp[:], pqp[:])
            pscp = pmsc.tile([1, T], F32, tag="pst")
            nc.tensor.matmul(pscp[:], qp[:], tKT_s[:], start=True, stop=False)
            nc.tensor.matmul(pscp[:], on_s[:, :1], bqK_s[:],
                             start=False, stop=True)
            mxp = smp.tile([1, 1], F32, tag="mxp")
            nc.vector.tensor_reduce(mxp[:], pscp[:], mybir.AxisListType.X, ALU.max)
            mxsp = smp.tile([1, 1], F32, tag="mxsp")
            nc.gpsimd.tensor_scalar(mxsp[:], mxp[:], -ISQ_TD, None, ALU.mult)
            exp_ = smp.tile([1, T], F32, tag="exp")
            nc.scalar.activation(exp_[:], pscp[:], AF.Exp, bias=mxsp[:],
                                 scale=ISQ_TD)
            smp_ = smp.tile([1, 1], F32, tag="smp")
            nc.vector.tensor_reduce(smp_[:], exp_[:], mybir.AxisListType.X, ALU.add)
            rcp = smp.tile([1, 1], F32, tag="rcp")
            nc.vector.reciprocal(rcp[:], smp_[:])
            wtsp = smp.tile([1, T], F32, tag="wtsp")
            nc.gpsimd.tensor_scalar(wtsp[:], exp_[:], rcp[:], None, ALU.mult)
            pwTp = pmsc.tile([T, 1], F32, tag="pmisc")
            nc.tensor.transpose(pwTp[:], wtsp[:], id_s[0:1, 0:1])
            wTp = smp.tile([T, 1], F32, tag="wTp")
            nc.vector.tensor_copy(wTp[:], pwTp[:])
            pnsp = pmsc.tile([128, 1], F32, tag="pmisc")
            nc.tensor.matmul(pnsp[:], tV_s[:], wTp[:], start=True, stop=True)
            nsp = smp.tile([128, 1], F32, tag="nsp")
            nc.vector.tensor_copy(nsp[:], pnsp[:])

            # ---- per-graph pooling with pad correction
            gembT = cp.tile([D, GPC], F32)
            strT = cp.tile([D, GPC], F32)
            for g in range(GPC):
                gs = slice(g * WPG, (g + 1) * WPG)
                for src, padc_col, dst in ((wsum_s, vp, gembT), (nsum_s, nsp, strT)):
                    tot = smp.tile([128, 1], F32, tag="tot")
                    nc.vector.tensor_reduce(tot[:], src[:, gs],
                                            mybir.AxisListType.X, ALU.add)
                    corr = smp.tile([128, 1], F32, tag="corr")
                    nc.gpsimd.tensor_tensor(corr[:], padc_col[:],
                                            pc_s[:, g:g + 1], ALU.mult)
                    t2 = smp.tile([128, 1], F32, tag="t2")
                    nc.vector.tensor_tensor(t2[:], tot[:], corr[:], ALU.subtract)
                    nc.vector.tensor_tensor(dst[:, g:g + 1], t2[:],
                                            ic_s[:, g:g + 1], ALU.mult)

            # ---- gating logits
            pl = pmsc.tile([NE, GPC], F32, tag="pmisc")
            nc.tensor.matmul(pl[:], Wg_s[:, 0, :], gembT[:], start=True, stop=False)
            nc.tensor.matmul(pl[:], Wg_s[:, 1, :], strT[:], start=False, stop=True)
            pa_ = pmsc.tile([NE, 1], F32, tag="pst")
            nc.tensor.matmul(pa_[:], on_s[:, :NE], al_s[:], start=True, stop=True)
            acol = smp.tile([NE, 1], F32, tag="acol")
            nc.vector.tensor_copy(acol[:], pa_[:])
            lg1 = smp.tile([NE, GPC], F32, tag="lg1")
            nc.vector.tensor_scalar(lg1[:], pl[:], bg_s[:], None, ALU.add)
            lg2 = smp.tile([NE, GPC], F32, tag="lg2")
            nc.vector.tensor_scalar(lg2[:], lg1[:], acol[:], 1.0 / TEMP,
                                    ALU.mult, ALU.mult)
            lg3 = smp.tile([NE, GPC], F32, tag="lg3")
            nc.vector.tensor_scalar(lg3[:], lg2[:], eb_s[:], None, ALU.add)
            nc.sync.dma_start(logitsT_o[:], lg3[:])

    nc.compile()
    return nc


# ------------------------------------------------------------- build kernel2

NSLOT = GPC * TOPK          # 16 dedicated (graph, k) slots per core
NCH = GPC + KS * GPC        # chunk-slots: 16 ded are per-graph; shared 2x4


def _build_k2():
    nc = bacc.Bacc("TRN2", target_bir_lowering=False, debug=False,
                   num_devices=NCORE)

    def din(name, shape, dt=F32):
        return nc.dram_tensor(name, shape, dt, kind="ExternalInput")

    vembT_i = din("vembT", [D, NC_NODES])
    logits_i = din("logits_nm", [GPC, NE])
    mask_i = din("mask_nm", [GPC, NE])
    Esel_i = din("Esel", [NSLOT, NE])
    Gsel_i = din("Gsel", [GPC, NSLOT])
    W1sel_i = din("W1sel", [D, NSLOT, 4 * D])
    b1selT_i = din("b1selT", [128, NSLOT * 4])
    W2hsel_i = din("W2hsel", [128, NSLOT, 4, D])
    b2sel_i = din("b2sel_row", [1, NSLOT * D])
    dgT_i = din("dgT", [D, NSLOT])
    dbbT_i = din("dbbT", [D, NSLOT])
    sW1_i = din("sW1T", [D, KS, 4 * D])
    sb1T_i = din("sb1T", [128, KS * 4])
    sW2h_i = din("sW2h", [128, KS, 4, D])
    sb2_i = din("sb2_row", [1, KS * D])
    sgT_i = din("sgT", [D, KS])
    sbbT_i = din("sbbT", [D, KS])
    hW1_i = din("hW1", [D, D])
    hb1_i = din("hb1_col", [D, 1])
    hW2_i = din("hW2col", [D, 1])
    hb2_i = din("hb2", [1, 1])
    ident_i = din("ident", [128, 128])
    onesr_i = din("onesr", [1, 512])

    out_o = nc.dram_tensor("out_row", [1, NC_NODES], F32, kind="ExternalOutput")

    HF = PAD_G // 2  # 384, half-chunk free dim

    with tile.TileContext(nc) as tc:
        with (
            tc.tile_pool(name="const", bufs=1) as cp,
            tc.tile_pool(name="wk", bufs=3) as wk,
            tc.tile_pool(name="hTp", bufs=2) as hTp,
            tc.tile_pool(name="sm", bufs=4) as smp,
            tc.tile_pool(name="ph", bufs=2, space="PSUM") as php,
            tc.tile_pool(name="pc", bufs=1, space="PSUM") as pcp,
            tc.tile_pool(name="pmc", bufs=2, space="PSUM") as pmcp,
        ):
            _ld = [0]
            def load(ap_dram, shape, dt=F32):
                _ld[0] += 1
                t_ = cp.tile(shape, dt, tag=f"cst{_ld[0]}")
                nc.sync.dma_start(t_[:], ap_dram[:])
                return t_

            vembT = load(vembT_i, [D, NC_NODES])
            acc = cp.tile([D, NC_NODES], F32)
            nc.sync.dma_start(acc[:], vembT_i[:])
            lgn = load(logits_i, [GPC, NE])
            msk = load(mask_i, [GPC, NE])
            Esel = load(Esel_i, [NSLOT, NE])
            Gsel = load(Gsel_i, [GPC, NSLOT])
            W1 = load(W1sel_i, [D, NSLOT, 4 * D])
            b1T = load(b1selT_i, [128, NSLOT * 4])
            W2 = load(W2hsel_i, [128, NSLOT, 4, D])
            b2r = load(b2sel_i, [1, NSLOT * D])
            dgT = load(dgT_i, [D, NSLOT])
            dbbT = load(dbbT_i, [D, NSLOT])
            sW1 = load(sW1_i, [D, KS, 4 * D])
            sb1T = load(sb1T_i, [128, KS * 4])
            sW2 = load(sW2h_i, [128, KS, 4, D])
            sb2r = load(sb2_i, [1, KS * D])
            sgT = load(sgT_i, [D, KS])
            sbbT = load(sbbT_i, [D, KS])
            hW1 = load(hW1_i, [D, D])
            hb1 = load(hb1_i, [D, 1])
            hW2 = load(hW2_i, [D, 1])
            hb2 = load(hb2_i, [1, 1])
            idn = load(ident_i, [128, 128])
            onr = load(onesr_i, [1, 512])
            ones_col = cp.tile([128, 1], F32)
            nc.vector.memset(ones_col[:], 1.0)
            eps11 = cp.tile([1, 1], F32)
            nc.vector.memset(eps11[:], LN_EPS)

            # ---- center W2 rows and b2 (W2 @ P done as W2 - rowmean(W2))
            for s in range(NSLOT + KS):
                for c in range(4):
                    wt = W2[:, s, c, :] if s < NSLOT else sW2[:, s - NSLOT, c, :]
                    mcol = smp.tile([128, 1], F32, tag="mcol")
                    nc.vector.tensor_reduce(mcol[:], wt, mybir.AxisListType.X,
                                            ALU.add)
                    m2 = smp.tile([128, 1], F32, tag="m2")
                    nc.gpsimd.tensor_scalar(m2[:], mcol[:], 1.0 / D, None, ALU.mult)
                    nc.gpsimd.tensor_scalar(wt, wt, m2[:], None, ALU.subtract)
                br = b2r[:, s * D:(s + 1) * D] if s < NSLOT else \
                    sb2r[:, (s - NSLOT) * D:(s - NSLOT + 1) * D]
                mb = smp.tile([1, 1], F32, tag="mb")
                nc.vector.tensor_reduce(mb[:], br, mybir.AxisListType.X, ALU.add)
                mb2 = smp.tile([1, 1], F32, tag="mb2")
                nc.gpsimd.tensor_scalar(mb2[:], mb[:], 1.0 / D, None, ALU.mult)
                nc.gpsimd.tensor_scalar(br, br, mb2[:], None, ALU.subtract)

            # ---- route weights on device
            mx = smp.tile([GPC, 1], F32, tag="mx")
            nc.vector.tensor_reduce(mx[:], lgn[:], mybir.AxisListType.X, ALU.max)
            nmx = smp.tile([GPC, 1], F32, tag="nmx")
            nc.gpsimd.tensor_scalar(nmx[:], mx[:], -1.0, None, ALU.mult)
            ex = smp.tile([GPC, NE], F32, tag="ex")
            nc.scalar.activation(ex[:], lgn[:], AF.Exp, bias=nmx[:])
            # full softmax then mask (denominator = sum over ALL experts)
            sme = smp.tile([GPC, 1], F32, tag="sme")
            nc.vector.tensor_reduce(sme[:], ex[:], mybir.AxisListType.X, ALU.add)
            rce = smp.tile([GPC, 1], F32, tag="rce")
            nc.vector.reciprocal(rce[:], sme[:])
            w_sm = smp.tile([GPC, NE], F32, tag="w_sm")
            nc.vector.tensor_scalar(w_sm[:], ex[:], rce[:], None, ALU.mult)
            wm = smp.tile([GPC, NE], F32, tag="wm")
            nc.vector.tensor_tensor(wm[:], w_sm[:], msk[:], ALU.mult)
            s2_ = smp.tile([GPC, 1], F32, tag="s2_")
            nc.vector.tensor_reduce(s2_[:], wm[:], mybir.AxisListType.X, ALU.add)
            s2e = smp.tile([GPC, 1], F32, tag="s2e")
            nc.gpsimd.tensor_scalar(s2e[:], s2_[:], 1e-12, None, ALU.add)
            rc2 = smp.tile([GPC, 1], F32, tag="rc2")
            nc.vector.reciprocal(rc2[:], s2e[:])
            route = smp.tile([GPC, NE], F32, tag="route")
            nc.vector.tensor_scalar(route[:], wm[:], rc2[:], None, ALU.mult)

            pR2 = pmcp.tile([NSLOT, NE], F32, tag="mc")
            nc.tensor.matmul(pR2[:], Gsel[:], route[:], start=True, stop=True)
            r2e = smp.tile([NSLOT, NE], F32, tag="r2e")
            nc.vector.tensor_tensor(r2e[:], pR2[:], Esel[:], ALU.mult)
            wc16 = smp.tile([NSLOT, 1], F32, tag="wc16")
            nc.vector.tensor_reduce(wc16[:], r2e[:], mybir.AxisListType.X, ALU.add)
            pwr = pmcp.tile([1, NSLOT], F32, tag="mc")
            nc.tensor.transpose(pwr[:], wc16[:], idn[:NSLOT, :NSLOT])
            wrow = cp.tile([1, NSLOT], F32)
            nc.vector.tensor_copy(wrow[:], pwr[:])

            # per-slot scale cols / bias cols
            wg_cols = cp.tile([D, NSLOT + KS], F32)
            wbb_cols = cp.tile([D, NSLOT + KS], F32)
            for s in range(NSLOT):
                pwb = pmcp.tile([128, 1], F32, tag="mc")
                nc.tensor.matmul(pwb[:], onr[:, :128], wrow[:, s:s + 1],
                                 start=True, stop=True)
                wbc = smp.tile([128, 1], F32, tag="wbc")
                nc.vector.tensor_copy(wbc[:], pwb[:])
                nc.vector.tensor_tensor(wg_cols[:, s:s + 1], dgT[:, s:s + 1],
                                        wbc[:], ALU.mult)
                nc.vector.tensor_tensor(wbb_cols[:, s:s + 1], dbbT[:, s:s + 1],
                                        wbc[:], ALU.mult)
            for s in range(KS):
                nc.vector.tensor_scalar(wg_cols[:, NSLOT + s:NSLOT + s + 1],
                                        sgT[:, s:s + 1], 1.0 / KS, None, ALU.mult)
                nc.vector.tensor_scalar(wbb_cols[:, NSLOT + s:NSLOT + s + 1],
                                        sbbT[:, s:s + 1], 1.0 / KS, None, ALU.mult)

            # ---- expert chunk-slots
            def chunk(slot, off, W1t, b1t, W2t, b2row):
                hT = hTp.tile([128, 4, PAD_G], F32, tag="hT")
                for c in range(4):
                    ph = php.tile([128, 2, 512], F32, tag="ph")
                    for h in range(2):
                        nc.tensor.matmul(
                            ph[:, h, :HF], W1t[:, c * 128:(c + 1) * 128],
                            vembT[:, off + h * HF:off + (h + 1) * HF],
                            start=True, stop=True)
                    nc.scalar.activation(hT[:, c, :], ph[:, :, :HF], AF.Gelu,
                                         bias=b1t[:, c:c + 1])
                pc_ = pcp.tile([128, 2, 512], F32, tag="pc")
                for h in range(2):
                    for c in range(4):
                        nc.tensor.matmul(pc_[:, h, :HF], W2t[:, c, :],
                                         hT[:, c, h * HF:(h + 1) * HF],
                                         start=(c == 0), stop=False)
                    nc.tensor.matmul(pc_[:, h, :HF], b2row, onr[:, :HF],
                                     start=False, stop=True)
                sq = wk.tile([128, 2, HF], F32, tag="sq")
                nc.scalar.activation(sq[:], pc_[:, :, :HF], AF.Square)
                pst = pmcp.tile([1, 512], F32, tag="mc")
                pst2 = pmcp.tile([1, 512], F32, tag="mc")
                nc.tensor.matmul(pst[:, :HF], ones_col[:], sq[:, 0, :],
                                 start=True, stop=True)
                nc.tensor.matmul(pst2[:, :HF], ones_col[:], sq[:, 1, :],
                                 start=True, stop=True)
                sd = wk.tile([1, PAD_G], F32, tag="sd")
                nc.scalar.activation(sd[:, :HF], pst[:, :HF], AF.Sqrt,
                                     bias=eps11[:], scale=1.0 / D)
                nc.scalar.activation(sd[:, HF:], pst2[:, :HF], AF.Sqrt,
                                     bias=eps11[:], scale=1.0 / D)
                rstd = wk.tile([1, PAD_G], F32, tag="rstd")
                nc.vector.reciprocal(rstd[:], sd[:])
                wgc = wg_cols[:, slot:slot + 1]
                wbc = wbb_cols[:, slot:slot + 1]
                for h in range(2):
                    cw = wk.tile([128, HF], F32, tag="cwk")
                    nc.vector.tensor_scalar(cw[:], pc_[:, h, :HF], wgc, None,
                                            ALU.mult)
                    pA = pmcp.tile([128, 512], F32, tag="mc")
                    nc.tensor.matmul(pA[:, :HF], onr[:, :128],
                                     rstd[:, h * HF:(h + 1) * HF],
                                     start=True, stop=True)
                    u = wk.tile([128, HF], F32, tag="u")
                    nc.vector.tensor_tensor(u[:], cw[:], pA[:, :HF], ALU.mult)
                    asl = acc[:, off + h * HF:off + (h + 1) * HF]
                    nc.vector.scalar_tensor_tensor(asl, u[:], wbc, asl,
                                                   ALU.add, ALU.add)

            for g in range(GPC):
                for k in range(TOPK):
                    s = g * TOPK + k
                    chunk(s, g * PAD_G, W1[:, s, :], b1T[:, s * 4:(s + 1) * 4],
                          W2[:, s, :, :], b2r[:, s * D:(s + 1) * D])
            for s in range(KS):
                for cc in range(GPC):
                    chunk(NSLOT + s, cc * PAD_G, sW1[:, s, :],
                          sb1T[:, s * 4:(s + 1) * 4], sW2[:, s, :, :],
                          sb2r[:, s * D:(s + 1) * D])

            # ---- task head
            for cc in range(GPC):
                off = cc * PAD_G
                pr = php.tile([128, 2, 512], F32, tag="ph")
                for h in range(2):
                    nc.tensor.matmul(pr[:, h, :HF], hW1[:],
                                     acc[:, off + h * HF:off + (h + 1) * HF],
                                     start=True, stop=True)
                r_sb = wk.tile([128, PAD_G], F32, tag="rsb")
                nc.scalar.activation(r_sb[:], pr[:, :, :HF], AF.Relu,
                                     bias=hb1[:])
                po = pcp.tile([1, 2, 512], F32, tag="pc")
                for h in range(2):
                    nc.tensor.matmul(po[:, h, :HF], hW2[:],
                                     r_sb[:, h * HF:(h + 1) * HF],
                                     start=True, stop=False)
                    nc.tensor.matmul(po[:, h, :HF], hb2[:], onr[:, :HF],
                                     start=False, stop=True)
                ot = wk.tile([1, PAD_G], F32, tag="ot")
                nc.vector.tensor_copy(ot[:], po[:, :, :HF])
                nc.sync.dma_start(out_o[:, off:off + PAD_G], ot[:])

    nc.compile()
    return nc


# ------------------------------------------------------------------- driver

_CACHE = {}


def kernel(**inputs):
    return _run(inputs, trace=False)[0]


def timed_run(inputs):
    _, t1, t2 = _run(inputs, trace=True)
    return t1, t2


def _run(inputs, trace=False):
    inp = {k: np.asarray(v) for k, v in inputs.items()}
    f32 = lambda k: inp[k].astype(np.float32)
    i64 = lambda k: inp[k].astype(np.int64)

    edge_cons, edge_vars, batch_idx = i64("edge_cons"), i64("edge_vars"), i64("batch_idx")
    plan = _plan(edge_cons, edge_vars, f32("edge_attr"), batch_idx)
    CW = tuple(plan["CW"])

    skip_bc = bool(np.all(inp["bc"] == 0))
    skip_be = bool(np.all(inp["be"] == 0))

    key1 = ("k1", CW, skip_bc, skip_be)
    if key1 not in _CACHE:
        _CACHE[key1] = _build_k1(list(CW), skip_bc, skip_be)
    nc1 = _CACHE[key1]

    iota = np.tile(np.arange(128, dtype=np.float32), (128, 1))
    ident = np.eye(128, dtype=np.float32)
    P_mat = (np.eye(128) - 1.0 / 128).astype(np.float32)
    onesr = np.ones((1, 512), np.float32)

    c_feat = f32("c_feat")
    v_feat = f32("v_feat")
    counts = plan["counts"]

    in1 = []
    for c in range(NCORE):
        nos = plan["node_of_slot"][c]
        vfT = np.zeros((VF, NC_NODES), np.float32)
        real = nos >= 0
        vfT[:, real] = v_feat[nos[real]].T
        cnt = counts[c].astype(np.float32)
        padc = (PAD_G - counts[c]).astype(np.float32)
        ecidx = plan["ecidx"][c]
        used = plan["used"][c]
        cfa = np.zeros((128 * plan["ntot"], CF + 1), np.float32)
        cfa[used, :CF] = c_feat[ecidx[used]]
        cfa[used, CF] = 1.0
        ntot = plan["ntot"]
        m = dict(
            edgecf=np.ascontiguousarray(
                cfa.reshape(ntot, 128, CF + 1).transpose(1, 0, 2).reshape(
                    128, ntot * (CF + 1))),
            Wc_aug=np.concatenate([f32("Wc"), f32("bc").reshape(1, D)], axis=0),
            Wv=f32("Wv"), bv_col=f32("bv").reshape(D, 1),
            vfeatT=vfT,
            We_col=f32("We").reshape(D, 1), be_col=f32("be").reshape(D, 1),
            lng_col=f32("ln_g").reshape(D, 1), lnb_col=f32("ln_b").reshape(D, 1),
            Wq=f32("Wq"), bq_col=f32("bq").reshape(TD, 1),
            tokKT=np.ascontiguousarray(f32("tokK").T),
            tokV=f32("tokV"),
            Wg_r=np.ascontiguousarray(f32("Wg").reshape(2, D, NE).transpose(1, 0, 2)),
            bg_col=f32("bg").reshape(NE, 1), eb_col=f32("ebias").reshape(NE, 1),
            alpha11=f32("alpha").reshape(1, 1),
            iota=iota, ident=ident, P_mat=P_mat, onesr=onesr,
            vloc=np.ascontiguousarray(plan["vloc"][c].reshape(-1, 128).T),
            eav=np.ascontiguousarray(plan["eav"][c].reshape(-1, 128).T),
            invcnt=np.tile((1.0 / np.maximum(cnt, 1.0))[None, :], (128, 1)),
            padcnt=np.tile(padc[None, :], (128, 1)),
        )
        in1.append(m)

    res1 = run_bass_kernel_spmd(nc1, in1, CORE_IDS, trace=trace)

    logits = np.concatenate(
        [res1.results[c]["logitsT"].T for c in range(NCORE)], axis=0)  # [B, NE]
    top_idx = np.argsort(-logits, axis=1, kind="stable")[:, :TOPK]     # [B, 4]
    mask = np.zeros((B, NE), np.float32)
    np.put_along_axis(mask, top_idx, 1.0, axis=1)

    if "k2" not in _CACHE:
        _CACHE["k2"] = _build_k2()
    nc2 = _CACHE["k2"]

    dW1, dW2 = f32("dW1"), f32("dW2")
    dg, dbb = f32("dg"), f32("dbb")
    sW1, sW2 = f32("sW1"), f32("sW2")
    Gsel = np.zeros((GPC, NSLOT), np.float32)
    for s in range(NSLOT):
        Gsel[s // TOPK, s] = 1.0

    in2 = []
    for c in range(NCORE):
        sel = top_idx[c * GPC:(c + 1) * GPC].reshape(-1)  # 16 expert ids
        Esel = np.zeros((NSLOT, NE), np.float32)
        Esel[np.arange(NSLOT), sel] = 1.0
        W1s = dW1[sel]                                  # [16, 128, 512]
        W2s = dW2[sel]                                  # [16, 512, 128]
        b1s = f32("db1")[sel]                           # [16, 512]
        b2s = f32("db2")[sel]                           # [16, 128]
        m = dict(
            vembT=res1.results[c]["vembT"],
            logits_nm=logits[c * GPC:(c + 1) * GPC],
            mask_nm=mask[c * GPC:(c + 1) * GPC],
            Esel=Esel, Gsel=Gsel,
            W1sel=np.ascontiguousarray(W1s.transpose(1, 0, 2)),
            b1selT=np.ascontiguousarray(
                b1s.reshape(NSLOT, 4, 128).transpose(2, 0, 1).reshape(128, NSLOT * 4)),
            W2hsel=np.ascontiguousarray(
                W2s.reshape(NSLOT, 4, 128, 128).transpose(2, 0, 1, 3)),
            b2sel_row=b2s.reshape(1, NSLOT * D),
            dgT=np.ascontiguousarray(dg[sel].T),
            dbbT=np.ascontiguousarray(dbb[sel].T),
            sW1T=np.ascontiguousarray(sW1.transpose(1, 0, 2)),
            sb1T=np.ascontiguousarray(
                f32("sb1").reshape(KS, 4, 128).transpose(2, 0, 1).reshape(128, KS * 4)),
            sW2h=np.ascontiguousarray(
                sW2.reshape(KS, 4, 128, 128).transpose(2, 0, 1, 3)),
            sb2_row=f32("sb2").reshape(1, KS * D),
            sgT=np.ascontiguousarray(f32("sg").T),
            sbbT=np.ascontiguousarray(f32("sbb").T),
            hW1=f32("hW1"), hb1_col=f32("hb1").reshape(D, 1),
            hW2col=f32("hW2").reshape(D, 1), hb2=f32("hb2").reshape(1, 1),
            ident=ident, onesr=onesr,
        )
        in2.append(m)

    res2 = run_bass_kernel_spmd(nc2, in2, CORE_IDS, trace=trace)

    out = np.zeros(N, np.float32)
    for c in range(NCORE):
        row = res2.results[c]["out_row"].reshape(-1)
        nos = plan["node_of_slot"][c]
        real = nos >= 0
        out[nos[real]] = row[real]
    return out, res1.exec_time_ns, res2.exec_time_ns
